# revision 20
# baseline (speedup 1.0000x reference)
"""Per-pixel dynamic 5x5 convolution (ApplyKernels) on 8 TRN2 NeuronCores.

Data-parallel over batch (8 batch elements -> 8 cores). Per core:
  kernel [25, 508, 508] f32, tensor [3, 512, 512] f32
  out    [4, 508, 508] f32  (3 weighted channels + kernel_sum)

Per core algorithm:
  - 5 row-blocks: output rows [124*ob, 124*ob+123] (last block: 12 rows).
    Input tile ob holds tensor rows [124*ob, 124*ob+127] (<=128 partitions),
    covering all 5 row taps (di in 0..4) with no cross-tile halo.
  - K tap-tiles are DMA-loaded with the row shift -di baked into the source
    offset, so the DVE product for tap (di,dj) is partition-aligned:
        P[p, c, j] = K[t, r0+p-di, j] * T[c, r0+p, j+dj]
  - PE accumulates taps into PSUM with a shifted-identity stationary
    W_di[p, po] = 1 iff po == p-di, undoing the row shift:
        psum[po] += P[po+di] -> weighted[r0+po]
    A 4th PSUM stream accumulates the K tiles themselves -> kernel_sum.
  - bf16 products (DVE tensor_tensor 2x mode); f32 accumulation in PSUM.
  - K staging slots are zeroed once at start; edge-block garbage partitions
    then always hold finite stale data, which the zero rows of W_di ignore.
"""

import numpy as np

_NC_CACHE = {}

KS = 5
NT = 25
R = 508
WIN = 512
CI = 3
NSTREAM = CI + 1
BLOCKS = [(0, 124), (124, 124), (248, 124), (372, 124), (496, 12)]
STG_BUFS = 6


def _build_nc():
    import concourse.bacc as bacc
    import concourse.mybir as mybir
    from concourse.tile import TileContext

    F32 = mybir.dt.float32
    BF16 = mybir.dt.bfloat16
    MULT = mybir.AluOpType.mult

    nc = bacc.Bacc("TRN2", target_bir_lowering=False, debug=False)
    Kp = nc.declare_dram_parameter("k", [NT, R, R], F32, isOutput=False)
    Tp = nc.declare_dram_parameter("t", [CI, WIN, WIN], F32, isOutput=False)
    Op = nc.declare_dram_parameter("out", [NSTREAM, R, R], F32, isOutput=True)

    with TileContext(nc) as tc:
        with (
            tc.tile_pool(name="const", bufs=1) as cpool,
            tc.tile_pool(name="tpool", bufs=2) as tpool,
            tc.tile_pool(name="kf32", bufs=STG_BUFS) as spool,
            tc.tile_pool(name="kbf", bufs=2) as kpool,
            tc.tile_pool(name="prod", bufs=8) as ppool,
            tc.tile_pool(name="ostage", bufs=8) as opool,
            tc.tile_pool(name="psum", bufs=2, space="PSUM") as psum_pool,
        ):
            # Shifted-identity stationaries W_di (bf16), one 128x128 per di.
            Wst = cpool.tile([128, KS * 128], BF16)
            nc.gpsimd.memset(Wst[:128], 0.0)
            for di in range(KS):
                sl = Wst[:128, di * 128 : (di + 1) * 128]
                nc.gpsimd.affine_select(
                    out=sl,
                    in_=sl,
                    compare_op=mybir.AluOpType.not_equal,
                    fill=1.0,
                    base=-di,
                    channel_multiplier=1,
                    pattern=[[-1, 128]],
                )

            def load_T(ob):
                # T block: f32 via ACT HWDGE queue, bf16 casts on DVE
                # (even + odd column parity keeps bf16 reads 4B-aligned).
                r0, rows = BLOCKS[ob]
                prows = min(rows + KS - 1, WIN - r0)
                tf32 = tpool.tile([128, CI * WIN], F32, name="tf32")
                nc.scalar.dma_start(
                    out=tf32[0:prows].rearrange("p (c j) -> p c j", c=CI),
                    in_=Tp[:, r0 : r0 + prows, :].transpose([1, 0, 2]),
                )
                te = tpool.tile([128, CI * WIN], BF16, name="te")
                to = tpool.tile([128, CI * WIN], BF16, name="to")
                nc.vector.tensor_copy(out=te[0:prows], in_=tf32[0:prows])
                nc.vector.tensor_copy(
                    out=to[0:prows].rearrange("p (c j) -> p c j", c=CI)[:, :, 0 : WIN - 1],
                    in_=tf32[0:prows].rearrange("p (c j) -> p c j", c=CI)[:, :, 1:WIN],
                )
                return te, to

            def load_K(ob):
                # K: f32 loads (row-shifted per di group) on sync HWDGE,
                # split at the partition-64 port boundary (the two halves use
                # disjoint SBUF port sets and overlap in the DMA fabric);
                # bf16 cast on ACT.
                r0, rows = BLOCKS[ob]
                prows = min(rows + KS - 1, WIN - r0)
                kbf = kpool.tile([128, NT * R], BF16, name="kbf")
                for di in range(KS):
                    stg = spool.tile([128, KS * R], F32, tag="stg", name="stg")
                    lo = r0 - di
                    p_a = max(0, -lo)
                    src_a = lo + p_a
                    src_b = min(R, lo + prows)
                    if ob == 0:
                        nc.gpsimd.memset(stg[0:4], 0.0)
                    elif ob == len(BLOCKS) - 1:
                        nc.gpsimd.memset(stg[0:16], 0.0)
                    for h0, h1 in ((p_a, min(64, p_a + (src_b - src_a))),
                                   (max(64, p_a), p_a + (src_b - src_a))):
                        if h1 <= h0:
                            continue
                        nc.sync.dma_start(
                            out=stg[h0:h1].rearrange("p (dj j) -> p dj j", dj=KS),
                            in_=Kp[
                                KS * di : KS * di + KS,
                                src_a + (h0 - p_a) : src_a + (h1 - p_a),
                                :,
                            ].transpose([1, 0, 2]),
                        )
                    nc.scalar.copy(
                        out=kbf[0:prows, KS * di * R : (KS * di + KS) * R],
                        in_=stg[0:prows],
                    )
                return kbf

            # Process the 12-row tail block FIRST: its K load is ~8x smaller
            # than a full block but its compute is full-width, so it fills
            # the pipeline while the big K streams ramp up.
            ORDER = [len(BLOCKS) - 1] + list(range(len(BLOCKS) - 1))
            pending = {ORDER[0]: (load_T(ORDER[0]), load_K(ORDER[0]))}
            for oi, ob in enumerate(ORDER):
                r0, rows = BLOCKS[ob]
                prows = min(rows + KS - 1, WIN - r0)  # 128 or 16
                (te, to), kbf = pending.pop(ob)
                # prefetch next block's inputs: these sit ahead of this
                # block's PSUM evacuation in the ACT/DVE FIFOs, so the next
                # block's products can start as soon as this block's finish.
                if oi + 1 < len(ORDER):
                    nxt = ORDER[oi + 1]
                    pending[nxt] = (load_T(nxt), load_K(nxt))

                ps = [
                    psum_pool.tile([128, R], mybir.dt.float32, tag=f"ps{s}", name=f"ps{s}")
                    for s in range(NSTREAM)
                ]

                for di in range(KS):
                    lhsT = Wst[0:prows, di * 128 : di * 128 + rows]
                    for dj in range(KS):
                        t_idx = KS * di + dj
                        P = ppool.tile([128, CI * R], BF16, tag="P", name="P")
                        ksl = kbf[0:prows, t_idx * R : (t_idx + 1) * R]
                        kb = ksl.unsqueeze(1).broadcast_to([prows, CI, R])
                        if dj % 2 == 0:
                            tv = te[0:prows].rearrange("p (c j) -> p c j", c=CI)[
                                :, :, dj : dj + R
                            ]
                        else:
                            tv = to[0:prows].rearrange("p (c j) -> p c j", c=CI)[
                                :, :, dj - 1 : dj - 1 + R
                            ]
                        nc.vector.tensor_tensor(
                            out=P[0:prows].rearrange("p (c j) -> p c j", c=CI),
                            in0=kb,
                            in1=tv,
                            op=MULT,
                        )
                        start = t_idx == 0
                        stop = t_idx == NT - 1
                        # ksum matmul first: it only needs kbf, so the PE can
                        # make progress while the DVE product is still running.
                        nc.tensor.matmul(
                            ps[CI][0:rows, :], lhsT, ksl, start=start, stop=stop
                        )
                        for s in range(CI):
                            rhs = P[0:prows, s * R : (s + 1) * R]
                            nc.tensor.matmul(
                                ps[s][0:rows, :], lhsT, rhs, start=start, stop=stop
                            )

                for s in range(NSTREAM):
                    ost = opool.tile([128, R], mybir.dt.float32)
                    nc.scalar.copy(out=ost[0:rows], in_=ps[s][0:rows])
                    nc.scalar.dma_start(out=Op[s, r0 : r0 + rows, :], in_=ost[0:rows])

    nc.compile()
    return nc


def kernel(**inputs):
    from concourse.bass_utils import run_bass_kernel_spmd

    k_full = np.ascontiguousarray(np.asarray(inputs["kernel"], dtype=np.float32))
    t_full = np.ascontiguousarray(np.asarray(inputs["tensor"], dtype=np.float32))
    bs = k_full.shape[0]
    assert k_full.shape == (bs, NT, R, R), k_full.shape
    assert t_full.shape == (bs, CI, WIN, WIN), t_full.shape

    if "nc" not in _NC_CACHE:
        _NC_CACHE["nc"] = _build_nc()
    nc = _NC_CACHE["nc"]

    core_ids = list(range(bs))
    in_maps = [{"k": k_full[b], "t": t_full[b]} for b in range(bs)]
    res = run_bass_kernel_spmd(nc, in_maps, core_ids=core_ids).results

    weighted = np.stack([res[b]["out"][:CI] for b in range(bs)], axis=0)
    ksum = np.stack([res[b]["out"][CI : CI + 1] for b in range(bs)], axis=0)
    return weighted.astype(np.float32), ksum.astype(np.float32)


# revision 21
# speedup vs baseline: 1.0346x; 1.0346x over previous
"""Per-pixel dynamic 5x5 convolution (ApplyKernels) on 8 TRN2 NeuronCores.

Data-parallel over batch (8 batch elements -> 8 cores). Per core:
  kernel [25, 508, 508] f32, tensor [3, 512, 512] f32
  out    [4, 508, 508] f32  (3 weighted channels + kernel_sum)

Per core algorithm:
  - 5 row-blocks: output rows [124*ob, 124*ob+123] (last block: 12 rows).
    Input tile ob holds tensor rows [124*ob, 124*ob+127] (<=128 partitions),
    covering all 5 row taps (di in 0..4) with no cross-tile halo.
  - K tap-tiles are DMA-loaded with the row shift -di baked into the source
    offset, so the DVE product for tap (di,dj) is partition-aligned:
        P[p, c, j] = K[t, r0+p-di, j] * T[c, r0+p, j+dj]
  - PE accumulates taps into PSUM with a shifted-identity stationary
    W_di[p, po] = 1 iff po == p-di, undoing the row shift:
        psum[po] += P[po+di] -> weighted[r0+po]
    A 4th PSUM stream accumulates the K tiles themselves -> kernel_sum.
  - bf16 products (DVE tensor_tensor 2x mode); f32 accumulation in PSUM.
  - K staging slots are zeroed once at start; edge-block garbage partitions
    then always hold finite stale data, which the zero rows of W_di ignore.
"""

import numpy as np

_NC_CACHE = {}

KS = 5
NT = 25
R = 508
WIN = 512
CI = 3
NSTREAM = CI + 1
BLOCKS = [(0, 124), (124, 124), (248, 124), (372, 124), (496, 12)]
STG_BUFS = 6


def _build_nc():
    import concourse.bacc as bacc
    import concourse.mybir as mybir
    from concourse.tile import TileContext

    F32 = mybir.dt.float32
    BF16 = mybir.dt.bfloat16
    MULT = mybir.AluOpType.mult

    nc = bacc.Bacc("TRN2", target_bir_lowering=False, debug=False)
    Kp = nc.declare_dram_parameter("k", [NT, R, R], F32, isOutput=False)
    Tp = nc.declare_dram_parameter("t", [CI, WIN, WIN], F32, isOutput=False)
    Op = nc.declare_dram_parameter("out", [NSTREAM, R, R], F32, isOutput=True)

    with TileContext(nc) as tc:
        with (
            tc.tile_pool(name="const", bufs=1) as cpool,
            tc.tile_pool(name="tpool", bufs=2) as tpool,
            tc.tile_pool(name="kf32", bufs=STG_BUFS) as spool,
            tc.tile_pool(name="kbf", bufs=2) as kpool,
            tc.tile_pool(name="prod", bufs=8) as ppool,
            tc.tile_pool(name="ostage", bufs=8) as opool,
            tc.tile_pool(name="psum", bufs=2, space="PSUM") as psum_pool,
        ):
            # Shifted-identity stationaries W_di (bf16), one 128x128 per di.
            Wst = cpool.tile([128, KS * 128], BF16)
            nc.gpsimd.memset(Wst[:128], 0.0)
            for di in range(KS):
                sl = Wst[:128, di * 128 : (di + 1) * 128]
                nc.gpsimd.affine_select(
                    out=sl,
                    in_=sl,
                    compare_op=mybir.AluOpType.not_equal,
                    fill=1.0,
                    base=-di,
                    channel_multiplier=1,
                    pattern=[[-1, 128]],
                )

            def load_T(ob):
                # T block: f32 via ACT HWDGE queue, bf16 casts on DVE
                # (even + odd column parity keeps bf16 reads 4B-aligned).
                r0, rows = BLOCKS[ob]
                prows = min(rows + KS - 1, WIN - r0)
                tf32 = tpool.tile([128, CI * WIN], F32, name="tf32")
                nc.scalar.dma_start(
                    out=tf32[0:prows].rearrange("p (c j) -> p c j", c=CI),
                    in_=Tp[:, r0 : r0 + prows, :].transpose([1, 0, 2]),
                )
                te = tpool.tile([128, CI * WIN], BF16, name="te")
                to = tpool.tile([128, CI * WIN], BF16, name="to")
                nc.vector.tensor_copy(out=te[0:prows], in_=tf32[0:prows])
                nc.vector.tensor_copy(
                    out=to[0:prows].rearrange("p (c j) -> p c j", c=CI)[:, :, 0 : WIN - 1],
                    in_=tf32[0:prows].rearrange("p (c j) -> p c j", c=CI)[:, :, 1:WIN],
                )
                return te, to

            def load_K(ob):
                # K: f32 loads (row-shifted per di group) on sync HWDGE,
                # split at the partition-64 port boundary (the two halves use
                # disjoint SBUF port sets and overlap in the DMA fabric);
                # bf16 cast on ACT.
                r0, rows = BLOCKS[ob]
                prows = min(rows + KS - 1, WIN - r0)
                kbf = kpool.tile([128, NT * R], BF16, name="kbf")
                for di in range(KS):
                    stg = spool.tile([128, KS * R], F32, tag="stg", name="stg")
                    lo = r0 - di
                    p_a = max(0, -lo)
                    src_a = lo + p_a
                    src_b = min(R, lo + prows)
                    if ob == 0:
                        nc.gpsimd.memset(stg[0:4], 0.0)
                    elif ob == len(BLOCKS) - 1:
                        nc.gpsimd.memset(stg[0:16], 0.0)
                    for h0, h1 in ((p_a, min(64, p_a + (src_b - src_a))),
                                   (max(64, p_a), p_a + (src_b - src_a))):
                        if h1 <= h0:
                            continue
                        eng = nc.sync if h0 < 64 else nc.scalar
                        eng.dma_start(
                            out=stg[h0:h1].rearrange("p (dj j) -> p dj j", dj=KS),
                            in_=Kp[
                                KS * di : KS * di + KS,
                                src_a + (h0 - p_a) : src_a + (h1 - p_a),
                                :,
                            ].transpose([1, 0, 2]),
                        )
                    nc.scalar.copy(
                        out=kbf[0:prows, KS * di * R : (KS * di + KS) * R],
                        in_=stg[0:prows],
                    )
                return kbf

            # Process the 12-row tail block FIRST: its K load is ~8x smaller
            # than a full block but its compute is full-width, so it fills
            # the pipeline while the big K streams ramp up.
            ORDER = [len(BLOCKS) - 1] + list(range(len(BLOCKS) - 1))
            pending = {ORDER[0]: (load_T(ORDER[0]), load_K(ORDER[0]))}
            for oi, ob in enumerate(ORDER):
                r0, rows = BLOCKS[ob]
                prows = min(rows + KS - 1, WIN - r0)  # 128 or 16
                (te, to), kbf = pending.pop(ob)
                # prefetch next block's inputs: these sit ahead of this
                # block's PSUM evacuation in the ACT/DVE FIFOs, so the next
                # block's products can start as soon as this block's finish.
                if oi + 1 < len(ORDER):
                    nxt = ORDER[oi + 1]
                    pending[nxt] = (load_T(nxt), load_K(nxt))

                ps = [
                    psum_pool.tile([128, R], mybir.dt.float32, tag=f"ps{s}", name=f"ps{s}")
                    for s in range(NSTREAM)
                ]

                for di in range(KS):
                    lhsT = Wst[0:prows, di * 128 : di * 128 + rows]
                    for dj in range(KS):
                        t_idx = KS * di + dj
                        P = ppool.tile([128, CI * R], BF16, tag="P", name="P")
                        ksl = kbf[0:prows, t_idx * R : (t_idx + 1) * R]
                        kb = ksl.unsqueeze(1).broadcast_to([prows, CI, R])
                        if dj % 2 == 0:
                            tv = te[0:prows].rearrange("p (c j) -> p c j", c=CI)[
                                :, :, dj : dj + R
                            ]
                        else:
                            tv = to[0:prows].rearrange("p (c j) -> p c j", c=CI)[
                                :, :, dj - 1 : dj - 1 + R
                            ]
                        nc.vector.tensor_tensor(
                            out=P[0:prows].rearrange("p (c j) -> p c j", c=CI),
                            in0=kb,
                            in1=tv,
                            op=MULT,
                        )
                        start = t_idx == 0
                        stop = t_idx == NT - 1
                        # ksum matmul first: it only needs kbf, so the PE can
                        # make progress while the DVE product is still running.
                        nc.tensor.matmul(
                            ps[CI][0:rows, :], lhsT, ksl, start=start, stop=stop
                        )
                        for s in range(CI):
                            rhs = P[0:prows, s * R : (s + 1) * R]
                            nc.tensor.matmul(
                                ps[s][0:rows, :], lhsT, rhs, start=start, stop=stop
                            )

                for s in range(NSTREAM):
                    ost = opool.tile([128, R], mybir.dt.float32)
                    nc.scalar.copy(out=ost[0:rows], in_=ps[s][0:rows])
                    nc.scalar.dma_start(out=Op[s, r0 : r0 + rows, :], in_=ost[0:rows])

    nc.compile()
    return nc


def kernel(**inputs):
    from concourse.bass_utils import run_bass_kernel_spmd

    k_full = np.ascontiguousarray(np.asarray(inputs["kernel"], dtype=np.float32))
    t_full = np.ascontiguousarray(np.asarray(inputs["tensor"], dtype=np.float32))
    bs = k_full.shape[0]
    assert k_full.shape == (bs, NT, R, R), k_full.shape
    assert t_full.shape == (bs, CI, WIN, WIN), t_full.shape

    if "nc" not in _NC_CACHE:
        _NC_CACHE["nc"] = _build_nc()
    nc = _NC_CACHE["nc"]

    core_ids = list(range(bs))
    in_maps = [{"k": k_full[b], "t": t_full[b]} for b in range(bs)]
    res = run_bass_kernel_spmd(nc, in_maps, core_ids=core_ids).results

    weighted = np.stack([res[b]["out"][:CI] for b in range(bs)], axis=0)
    ksum = np.stack([res[b]["out"][CI : CI + 1] for b in range(bs)], axis=0)
    return weighted.astype(np.float32), ksum.astype(np.float32)


# revision 22
# speedup vs baseline: 1.1057x; 1.0688x over previous
"""Per-pixel dynamic 5x5 convolution (ApplyKernels) on 8 TRN2 NeuronCores.

Data-parallel over batch (8 batch elements -> 8 cores). Per core:
  kernel [25, 508, 508] f32, tensor [3, 512, 512] f32
  out    [4, 508, 508] f32  (3 weighted channels + kernel_sum)

Per core algorithm:
  - 5 row-blocks: output rows [124*ob, 124*ob+123] (last block: 12 rows).
    Input tile ob holds tensor rows [124*ob, 124*ob+127] (<=128 partitions),
    covering all 5 row taps (di in 0..4) with no cross-tile halo.
  - K tap-tiles are DMA-loaded with the row shift -di baked into the source
    offset, so the DVE product for tap (di,dj) is partition-aligned:
        P[p, c, j] = K[t, r0+p-di, j] * T[c, r0+p, j+dj]
  - PE accumulates taps into PSUM with a shifted-identity stationary
    W_di[p, po] = 1 iff po == p-di, undoing the row shift:
        psum[po] += P[po+di] -> weighted[r0+po]
    A 4th PSUM stream accumulates the K tiles themselves -> kernel_sum.
  - bf16 products (DVE tensor_tensor 2x mode); f32 accumulation in PSUM.
  - K staging slots are zeroed once at start; edge-block garbage partitions
    then always hold finite stale data, which the zero rows of W_di ignore.
"""

import numpy as np

_NC_CACHE = {}

KS = 5
NT = 25
R = 508
WIN = 512
CI = 3
NSTREAM = CI + 1
BLOCKS = [(0, 124), (124, 124), (248, 124), (372, 124), (496, 12)]
STG_BUFS = 6


def _build_nc():
    import concourse.bacc as bacc
    import concourse.mybir as mybir
    from concourse.tile import TileContext

    F32 = mybir.dt.float32
    BF16 = mybir.dt.bfloat16
    MULT = mybir.AluOpType.mult

    nc = bacc.Bacc("TRN2", target_bir_lowering=False, debug=False)
    Kp = nc.declare_dram_parameter("k", [NT, R, R], F32, isOutput=False)
    Tp = nc.declare_dram_parameter("t", [CI, WIN, WIN], F32, isOutput=False)
    Op = nc.declare_dram_parameter("out", [NSTREAM, R, R], F32, isOutput=True)

    with TileContext(nc) as tc:
        with (
            tc.tile_pool(name="const", bufs=1) as cpool,
            tc.tile_pool(name="tpool", bufs=2) as tpool,
            tc.tile_pool(name="kf32", bufs=STG_BUFS) as spool,
            tc.tile_pool(name="kbf", bufs=2) as kpool,
            tc.tile_pool(name="prod", bufs=8) as ppool,
            tc.tile_pool(name="ostage", bufs=8) as opool,
            tc.tile_pool(name="psum", bufs=2, space="PSUM") as psum_pool,
        ):
            # Shifted-identity stationaries W_di (bf16), one 128x128 per di.
            Wst = cpool.tile([128, KS * 128], BF16)
            nc.gpsimd.memset(Wst[:128], 0.0)
            for di in range(KS):
                sl = Wst[:128, di * 128 : (di + 1) * 128]
                nc.gpsimd.affine_select(
                    out=sl,
                    in_=sl,
                    compare_op=mybir.AluOpType.not_equal,
                    fill=1.0,
                    base=-di,
                    channel_multiplier=1,
                    pattern=[[-1, 128]],
                )

            def load_T(ob):
                # T block: f32 via ACT HWDGE queue, bf16 casts on DVE
                # (even + odd column parity keeps bf16 reads 4B-aligned).
                r0, rows = BLOCKS[ob]
                prows = min(rows + KS - 1, WIN - r0)
                tf32 = tpool.tile([128, CI * WIN], F32, name="tf32")
                nc.scalar.dma_start(
                    out=tf32[0:prows].rearrange("p (c j) -> p c j", c=CI),
                    in_=Tp[:, r0 : r0 + prows, :].transpose([1, 0, 2]),
                )
                te = tpool.tile([128, CI * WIN], BF16, name="te")
                to = tpool.tile([128, CI * WIN], BF16, name="to")
                nc.vector.tensor_copy(out=te[0:prows], in_=tf32[0:prows])
                nc.vector.tensor_copy(
                    out=to[0:prows].rearrange("p (c j) -> p c j", c=CI)[:, :, 0 : WIN - 1],
                    in_=tf32[0:prows].rearrange("p (c j) -> p c j", c=CI)[:, :, 1:WIN],
                )
                return te, to

            def load_K(ob):
                # K: f32 loads (row-shifted per di group) on sync HWDGE,
                # split at the partition-64 port boundary (the two halves use
                # disjoint SBUF port sets and overlap in the DMA fabric);
                # bf16 cast on ACT.
                r0, rows = BLOCKS[ob]
                prows = min(rows + KS - 1, WIN - r0)
                kbf = kpool.tile([128, NT * R], BF16, name="kbf")
                for di in range(KS):
                    stg = spool.tile([128, KS * R], F32, tag="stg", name="stg")
                    lo = r0 - di
                    p_a = max(0, -lo)
                    src_a = lo + p_a
                    src_b = min(R, lo + prows)
                    if ob == 0:
                        nc.gpsimd.memset(stg[0:4], 0.0)
                    elif ob == len(BLOCKS) - 1:
                        nc.gpsimd.memset(stg[0:16], 0.0)
                    for h0, h1 in ((p_a, min(64, p_a + (src_b - src_a))),
                                   (max(64, p_a), p_a + (src_b - src_a))):
                        if h1 <= h0:
                            continue
                        nc.sync.dma_start(
                            out=stg[h0:h1].rearrange("p (dj j) -> p dj j", dj=KS),
                            in_=Kp[
                                KS * di : KS * di + KS,
                                src_a + (h0 - p_a) : src_a + (h1 - p_a),
                                :,
                            ].transpose([1, 0, 2]),
                        )
                    nc.scalar.copy(
                        out=kbf[0:prows, KS * di * R : (KS * di + KS) * R],
                        in_=stg[0:prows],
                    )
                return kbf

            # Process the 12-row tail block FIRST: its K load is ~8x smaller
            # than a full block but its compute is full-width, so it fills
            # the pipeline while the big K streams ramp up.
            ORDER = [len(BLOCKS) - 1] + list(range(len(BLOCKS) - 1))
            pending = {ORDER[0]: (load_T(ORDER[0]), load_K(ORDER[0]))}
            for oi, ob in enumerate(ORDER):
                r0, rows = BLOCKS[ob]
                prows = min(rows + KS - 1, WIN - r0)  # 128 or 16
                (te, to), kbf = pending.pop(ob)
                # prefetch next block's inputs: these sit ahead of this
                # block's PSUM evacuation in the ACT/DVE FIFOs, so the next
                # block's products can start as soon as this block's finish.
                if oi + 1 < len(ORDER):
                    nxt = ORDER[oi + 1]
                    pending[nxt] = (load_T(nxt), load_K(nxt))

                ps = [
                    psum_pool.tile([128, R], mybir.dt.float32, tag=f"ps{s}", name=f"ps{s}")
                    for s in range(NSTREAM)
                ]

                for di in range(KS):
                    lhsT = Wst[0:prows, di * 128 : di * 128 + rows]
                    for dj in range(KS):
                        t_idx = KS * di + dj
                        P = ppool.tile([128, CI * R], BF16, tag="P", name="P")
                        ksl = kbf[0:prows, t_idx * R : (t_idx + 1) * R]
                        kb = ksl.unsqueeze(1).broadcast_to([prows, CI, R])
                        if dj % 2 == 0:
                            tv = te[0:prows].rearrange("p (c j) -> p c j", c=CI)[
                                :, :, dj : dj + R
                            ]
                        else:
                            tv = to[0:prows].rearrange("p (c j) -> p c j", c=CI)[
                                :, :, dj - 1 : dj - 1 + R
                            ]
                        nc.vector.tensor_tensor(
                            out=P[0:prows].rearrange("p (c j) -> p c j", c=CI),
                            in0=kb,
                            in1=tv,
                            op=MULT,
                        )
                        start = t_idx == 0
                        stop = t_idx == NT - 1
                        # ksum matmul first: it only needs kbf, so the PE can
                        # make progress while the DVE product is still running.
                        nc.tensor.matmul(
                            ps[CI][0:rows, :], lhsT, ksl, start=start, stop=stop
                        )
                        for s in range(CI):
                            rhs = P[0:prows, s * R : (s + 1) * R]
                            nc.tensor.matmul(
                                ps[s][0:rows, :], lhsT, rhs, start=start, stop=stop
                            )

                for s in range(NSTREAM):
                    ost = opool.tile([128, R], mybir.dt.float32)
                    nc.scalar.copy(out=ost[0:rows], in_=ps[s][0:rows])
                    nc.scalar.dma_start(out=Op[s, r0 : r0 + rows, :], in_=ost[0:rows])

    nc.compile()
    return nc


def kernel(**inputs):
    from concourse.bass_utils import run_bass_kernel_spmd

    k_full = np.ascontiguousarray(np.asarray(inputs["kernel"], dtype=np.float32))
    t_full = np.ascontiguousarray(np.asarray(inputs["tensor"], dtype=np.float32))
    bs = k_full.shape[0]
    assert k_full.shape == (bs, NT, R, R), k_full.shape
    assert t_full.shape == (bs, CI, WIN, WIN), t_full.shape

    if "nc" not in _NC_CACHE:
        _NC_CACHE["nc"] = _build_nc()
    nc = _NC_CACHE["nc"]

    core_ids = list(range(bs))
    in_maps = [{"k": k_full[b], "t": t_full[b]} for b in range(bs)]
    res = run_bass_kernel_spmd(nc, in_maps, core_ids=core_ids).results

    weighted = np.stack([res[b]["out"][:CI] for b in range(bs)], axis=0)
    ksum = np.stack([res[b]["out"][CI : CI + 1] for b in range(bs)], axis=0)
    return weighted.astype(np.float32), ksum.astype(np.float32)


# revision 23
# speedup vs baseline: 1.1199x; 1.0128x over previous
"""Per-pixel dynamic 5x5 convolution (ApplyKernels) on 8 TRN2 NeuronCores.

Data-parallel over batch (8 batch elements -> 8 cores). Per core:
  kernel [25, 508, 508] f32, tensor [3, 512, 512] f32
  out    [4, 508, 508] f32  (3 weighted channels + kernel_sum)

Per core algorithm:
  - 5 row-blocks: output rows [124*ob, 124*ob+123] (last block: 12 rows).
    Input tile ob holds tensor rows [124*ob, 124*ob+127] (<=128 partitions),
    covering all 5 row taps (di in 0..4) with no cross-tile halo.
  - K tap-tiles are DMA-loaded with the row shift -di baked into the source
    offset, so the DVE product for tap (di,dj) is partition-aligned:
        P[p, c, j] = K[t, r0+p-di, j] * T[c, r0+p, j+dj]
  - PE accumulates taps into PSUM with a shifted-identity stationary
    W_di[p, po] = 1 iff po == p-di, undoing the row shift:
        psum[po] += P[po+di] -> weighted[r0+po]
    A 4th PSUM stream accumulates the K tiles themselves -> kernel_sum.
  - bf16 products (DVE tensor_tensor 2x mode); f32 accumulation in PSUM.
  - K staging slots are zeroed once at start; edge-block garbage partitions
    then always hold finite stale data, which the zero rows of W_di ignore.
"""

import numpy as np

_NC_CACHE = {}

KS = 5
NT = 25
R = 508
WIN = 512
CI = 3
NSTREAM = CI + 1
BLOCKS = [(0, 124), (124, 124), (248, 124), (372, 124), (496, 12)]
STG_BUFS = 8


def _build_nc():
    import concourse.bacc as bacc
    import concourse.mybir as mybir
    from concourse.tile import TileContext

    F32 = mybir.dt.float32
    BF16 = mybir.dt.bfloat16
    MULT = mybir.AluOpType.mult

    nc = bacc.Bacc("TRN2", target_bir_lowering=False, debug=False)
    Kp = nc.declare_dram_parameter("k", [NT, R, R], F32, isOutput=False)
    Tp = nc.declare_dram_parameter("t", [CI, WIN, WIN], F32, isOutput=False)
    Op = nc.declare_dram_parameter("out", [NSTREAM, R, R], F32, isOutput=True)

    with TileContext(nc) as tc:
        with (
            tc.tile_pool(name="const", bufs=1) as cpool,
            tc.tile_pool(name="tpool", bufs=2) as tpool,
            tc.tile_pool(name="kf32", bufs=STG_BUFS) as spool,
            tc.tile_pool(name="kbf", bufs=2) as kpool,
            tc.tile_pool(name="prod", bufs=8) as ppool,
            tc.tile_pool(name="ostage", bufs=8) as opool,
            tc.tile_pool(name="psum", bufs=2, space="PSUM") as psum_pool,
        ):
            # Shifted-identity stationaries W_di (bf16), one 128x128 per di.
            Wst = cpool.tile([128, KS * 128], BF16)
            nc.gpsimd.memset(Wst[:128], 0.0)
            for di in range(KS):
                sl = Wst[:128, di * 128 : (di + 1) * 128]
                nc.gpsimd.affine_select(
                    out=sl,
                    in_=sl,
                    compare_op=mybir.AluOpType.not_equal,
                    fill=1.0,
                    base=-di,
                    channel_multiplier=1,
                    pattern=[[-1, 128]],
                )

            def load_T(ob):
                # T block: f32 via ACT HWDGE queue, bf16 casts on DVE
                # (even + odd column parity keeps bf16 reads 4B-aligned).
                r0, rows = BLOCKS[ob]
                prows = min(rows + KS - 1, WIN - r0)
                tf32 = tpool.tile([128, CI * WIN], F32, name="tf32")
                nc.scalar.dma_start(
                    out=tf32[0:prows].rearrange("p (c j) -> p c j", c=CI),
                    in_=Tp[:, r0 : r0 + prows, :].transpose([1, 0, 2]),
                )
                te = tpool.tile([128, CI * WIN], BF16, name="te")
                to = tpool.tile([128, CI * WIN], BF16, name="to")
                nc.vector.tensor_copy(out=te[0:prows], in_=tf32[0:prows])
                nc.vector.tensor_copy(
                    out=to[0:prows].rearrange("p (c j) -> p c j", c=CI)[:, :, 0 : WIN - 1],
                    in_=tf32[0:prows].rearrange("p (c j) -> p c j", c=CI)[:, :, 1:WIN],
                )
                return te, to

            def load_K(ob):
                # K: f32 loads (row-shifted per di group) on sync HWDGE,
                # split at the partition-64 port boundary (the two halves use
                # disjoint SBUF port sets and overlap in the DMA fabric);
                # bf16 cast on ACT.
                r0, rows = BLOCKS[ob]
                prows = min(rows + KS - 1, WIN - r0)
                kbf = kpool.tile([128, NT * R], BF16, name="kbf")
                for di in range(KS):
                    stg = spool.tile([128, KS * R], F32, tag="stg", name="stg")
                    lo = r0 - di
                    p_a = max(0, -lo)
                    src_a = lo + p_a
                    src_b = min(R, lo + prows)
                    if ob == 0:
                        nc.gpsimd.memset(stg[0:4], 0.0)
                    elif ob == len(BLOCKS) - 1:
                        nc.gpsimd.memset(stg[0:16], 0.0)
                    for h0, h1 in ((p_a, min(64, p_a + (src_b - src_a))),
                                   (max(64, p_a), p_a + (src_b - src_a))):
                        if h1 <= h0:
                            continue
                        nc.sync.dma_start(
                            out=stg[h0:h1].rearrange("p (dj j) -> p dj j", dj=KS),
                            in_=Kp[
                                KS * di : KS * di + KS,
                                src_a + (h0 - p_a) : src_a + (h1 - p_a),
                                :,
                            ].transpose([1, 0, 2]),
                        )
                    nc.scalar.copy(
                        out=kbf[0:prows, KS * di * R : (KS * di + KS) * R],
                        in_=stg[0:prows],
                    )
                return kbf

            # Process the 12-row tail block FIRST: its K load is ~8x smaller
            # than a full block but its compute is full-width, so it fills
            # the pipeline while the big K streams ramp up.
            ORDER = [len(BLOCKS) - 1] + list(range(len(BLOCKS) - 1))
            pending = {ORDER[0]: (load_T(ORDER[0]), load_K(ORDER[0]))}
            for oi, ob in enumerate(ORDER):
                r0, rows = BLOCKS[ob]
                prows = min(rows + KS - 1, WIN - r0)  # 128 or 16
                (te, to), kbf = pending.pop(ob)
                # prefetch next block's inputs: these sit ahead of this
                # block's PSUM evacuation in the ACT/DVE FIFOs, so the next
                # block's products can start as soon as this block's finish.
                if oi + 1 < len(ORDER):
                    nxt = ORDER[oi + 1]
                    pending[nxt] = (load_T(nxt), load_K(nxt))

                ps = [
                    psum_pool.tile([128, R], mybir.dt.float32, tag=f"ps{s}", name=f"ps{s}")
                    for s in range(NSTREAM)
                ]

                for di in range(KS):
                    lhsT = Wst[0:prows, di * 128 : di * 128 + rows]
                    for dj in range(KS):
                        t_idx = KS * di + dj
                        P = ppool.tile([128, CI * R], BF16, tag="P", name="P")
                        ksl = kbf[0:prows, t_idx * R : (t_idx + 1) * R]
                        kb = ksl.unsqueeze(1).broadcast_to([prows, CI, R])
                        if dj % 2 == 0:
                            tv = te[0:prows].rearrange("p (c j) -> p c j", c=CI)[
                                :, :, dj : dj + R
                            ]
                        else:
                            tv = to[0:prows].rearrange("p (c j) -> p c j", c=CI)[
                                :, :, dj - 1 : dj - 1 + R
                            ]
                        nc.vector.tensor_tensor(
                            out=P[0:prows].rearrange("p (c j) -> p c j", c=CI),
                            in0=kb,
                            in1=tv,
                            op=MULT,
                        )
                        start = t_idx == 0
                        stop = t_idx == NT - 1
                        # ksum matmul first: it only needs kbf, so the PE can
                        # make progress while the DVE product is still running.
                        nc.tensor.matmul(
                            ps[CI][0:rows, :], lhsT, ksl, start=start, stop=stop
                        )
                        for s in range(CI):
                            rhs = P[0:prows, s * R : (s + 1) * R]
                            nc.tensor.matmul(
                                ps[s][0:rows, :], lhsT, rhs, start=start, stop=stop
                            )

                for s in range(NSTREAM):
                    ost = opool.tile([128, R], mybir.dt.float32)
                    nc.scalar.copy(out=ost[0:rows], in_=ps[s][0:rows])
                    nc.scalar.dma_start(out=Op[s, r0 : r0 + rows, :], in_=ost[0:rows])

    nc.compile()
    return nc


def kernel(**inputs):
    from concourse.bass_utils import run_bass_kernel_spmd

    k_full = np.ascontiguousarray(np.asarray(inputs["kernel"], dtype=np.float32))
    t_full = np.ascontiguousarray(np.asarray(inputs["tensor"], dtype=np.float32))
    bs = k_full.shape[0]
    assert k_full.shape == (bs, NT, R, R), k_full.shape
    assert t_full.shape == (bs, CI, WIN, WIN), t_full.shape

    if "nc" not in _NC_CACHE:
        _NC_CACHE["nc"] = _build_nc()
    nc = _NC_CACHE["nc"]

    core_ids = list(range(bs))
    in_maps = [{"k": k_full[b], "t": t_full[b]} for b in range(bs)]
    res = run_bass_kernel_spmd(nc, in_maps, core_ids=core_ids).results

    weighted = np.stack([res[b]["out"][:CI] for b in range(bs)], axis=0)
    ksum = np.stack([res[b]["out"][CI : CI + 1] for b in range(bs)], axis=0)
    return weighted.astype(np.float32), ksum.astype(np.float32)
